# revision 1
# baseline (speedup 1.0000x reference)
"""KoLeo-loss kernel for Trainium2, 8 NeuronCores.

Math: rows are L2-normalized, so for unit vectors dist(a,b) = sqrt(2-2*a.b).
The per-row NN distance only needs the row-max of the (diagonal-masked) cosine
Gram matrix — no argmax / gather. loss_i = -log(sqrt(2-2*m_i)+1e-8)
= -0.5*ln(2-2*m_i) (the 1e-8 is below fp32 resolution at dist~1.3).

Sharding: replicate keys, shard the BxB Gram rows across 8 cores (512 rows
each). Each core returns the partial scalar sum(-0.5*ln t)/B over its rows and
both views; the host adds the 8 partials.

Per-core pipeline (all static code; per-core data = xq row-slice + one-hot
gsel):
  - stream 32 chunks [128 rows, 2 views, 1024] fp32 from HBM
  - ACT: Square+accum -> sumsq; Ln+Exp -> rinv = (ss+eps)^-0.5
  - DVE/GPSIMD: y = x * rinv  (bf16)
  - PE transpose (fp32 "pair-packed" bf16) -> YT[d-major] in SBUF
  - bf16 matmuls lhsT=own-rows-T, rhs=YT group slice, accumulated over 8
    K-phases into PSUM [128,512]; +1 mask matmul adds -4 on the diagonal
    (lhsT = -4*I*gsel[g], rhs = shifted-identity pattern)
  - DVE reduce_max per PSUM block; finale: -0.5*ln(2-2*max), partition-sum
    via ones-matmul, scalar out.
"""

import os
import sys
from contextlib import ExitStack

import numpy as np

sys.path.insert(0, "/opt/trn_rl_repo")

import concourse.bass as bass
import concourse.mybir as mybir
import concourse.tile as tile
from concourse import bacc, bass_utils

F32 = mybir.dt.float32
BF16 = mybir.dt.bfloat16
AF = mybir.ActivationFunctionType

B, V, D = 4096, 2, 1024
NCORES = 8
MB = B // NCORES          # 512 own rows per core
NCHUNK = B // 128         # 32 key chunks
NQ = MB // 128            # 4 own chunks
NG = 8                    # column groups of 512 keys
EPS = 1e-8
MASKV = -4.0


def _process_chunk(nc, pools, x_src, dstT, col0, norm_eng):
    """Load one [128, V, D] fp32 chunk, normalize to bf16, transpose into
    dstT[:, v*4+d2, col0:col0+128] (fp32 pair-packed layout).

    norm_eng: single engine writing the yb tile (single-writer keeps the
    downstream transpose-matmul within the HW sync-wait slot budget)."""
    xpool, ypool, sqpool, sspool, trp, identF, epsb = pools
    xt = xpool.tile([128, V, D], F32, tag="xraw", name="xraw")
    nc.sync.dma_start(xt[:], x_src)

    ss = sspool.tile([128, V], F32, tag="ss", name="ss")
    sq = sqpool.tile([128, D], BF16, tag="sq", name="sq")
    for v in range(V):
        nc.scalar.activation(sq[:], xt[:, v, :], AF.Square, accum_out=ss[:, v : v + 1])
    # rinv = sqrt(1/(ss+eps)): DVE reciprocal (no ACT table) + ACT Sqrt
    # (same filler table set as Square -> no ACT_TABLE_LOAD thrash per chunk)
    rec = sspool.tile([128, V], F32, tag="rec", name="rec")
    nc.vector.tensor_scalar_add(rec[:], ss[:], EPS)
    nc.vector.reciprocal(rec[:], rec[:])
    rinv = sspool.tile([128, V], F32, tag="rinv", name="rinv")
    nc.scalar.activation(rinv[:], rec[:], AF.Sqrt)

    # rloc: same-engine staging of rinv so the tensor_scalar below carries a
    # single cross-engine wait (xt/DMA); HW allows only one sync wait per op.
    rloc = sspool.tile([128, V], F32, tag="rloc", name="rloc")
    norm_eng.tensor_copy(rloc[:], rinv[:])
    yb = ypool.tile([128, V, D], BF16, tag="ybf", name="ybf")
    for v in range(V):
        norm_eng.tensor_scalar_mul(yb[:, v, :], xt[:, v, :], rloc[:, v : v + 1])

    ybF = yb.bitcast(F32)  # [128, V, 512]: word j packs y[.., 2j], y[.., 2j+1]
    for v in range(V):
        for d2 in range(4):
            tp = trp.tile([128, 128], F32, tag="tp", name="tp")
            nc.tensor.transpose(tp[:], ybF[:, v, 128 * d2 : 128 * (d2 + 1)], identF[:])
            nc.vector.tensor_copy(dstT[:, v * 4 + d2, col0 : col0 + 128], tp[:])


def build():
    nc = bacc.Bacc("TRN2", debug=False)
    x_d = nc.dram_tensor("x", [B, V, D], F32, kind="ExternalInput").ap()
    xq_d = nc.dram_tensor("xq", [MB, V, D], F32, kind="ExternalInput").ap()
    gs_d = nc.dram_tensor("gsel", [128, NG], F32, kind="ExternalInput").ap()
    out_d = nc.dram_tensor("out", [1, 1], F32, kind="ExternalOutput").ap()

    with ExitStack() as ctx:
        tc = ctx.enter_context(tile.TileContext(nc))
        const = ctx.enter_context(tc.tile_pool(name="const", bufs=1))
        xpool = ctx.enter_context(tc.tile_pool(name="xpool", bufs=3))
        ypool = ctx.enter_context(tc.tile_pool(name="ypool", bufs=3))
        sqpool = ctx.enter_context(tc.tile_pool(name="sqpool", bufs=2))
        sspool = ctx.enter_context(tc.tile_pool(name="sspool", bufs=3))
        accp = ctx.enter_context(tc.tile_pool(name="accp", bufs=3, space="PSUM"))
        trp = ctx.enter_context(tc.tile_pool(name="trp", bufs=3, space="PSUM"))
        smallp = ctx.enter_context(tc.tile_pool(name="smallp", bufs=2, space="PSUM"))

        # ---- constants ----
        identF = const.tile([128, 128], F32, name="identF")
        nc.gpsimd.memset(identF[:], 0.0)
        nc.gpsimd.affine_select(
            out=identF[:], in_=identF[:], compare_op=mybir.AluOpType.not_equal,
            fill=1.0, base=0, pattern=[[-1, 128]], channel_multiplier=1)

        negI = const.tile([128, 128], BF16, name="negI")
        nc.gpsimd.memset(negI[:], 0.0)
        nc.gpsimd.affine_select(
            out=negI[:], in_=negI[:], compare_op=mybir.AluOpType.not_equal,
            fill=MASKV, base=0, pattern=[[-1, 128]], channel_multiplier=1)

        # zpat[p, f] = 1 iff f == p + 384  (I-block at cols [384, 512))
        zpat = const.tile([128, 896], BF16, name="zpat")
        nc.gpsimd.memset(zpat[:], 0.0)
        nc.gpsimd.affine_select(
            out=zpat[:], in_=zpat[:], compare_op=mybir.AluOpType.not_equal,
            fill=1.0, base=384, pattern=[[-1, 896]], channel_multiplier=1)

        ones = const.tile([128, 1], F32, name="ones")
        nc.vector.memset(ones[:], 1.0)
        epsb = const.tile([128, 1], F32, name="epsb")
        nc.gpsimd.memset(epsb[:], EPS)

        # gsel arrives host-replicated [128, NG]; scale -4*I by gsel[g].
        # Built on gpsimd so negI is a same-engine read (single-wait rule).
        gsbc = const.tile([128, NG], F32, name="gsbc")
        nc.sync.dma_start(gsbc[:], gs_d)
        gselI = const.tile([128, NG, 128], BF16, name="gselI")
        for g in range(NG):
            nc.gpsimd.tensor_scalar_mul(gselI[:, g, :], negI[:], gsbc[:, g : g + 1])

        # Dummy transpose: advances PE's observed gpsimd clock past the const
        # writes, so later transposes/matmuls need only their one data wait.
        tpd = trp.tile([128, 128], F32, tag="tp", name="tpd")
        nc.tensor.transpose(tpd[:], identF[:], identF[:])

        # ---- persistent transposed buffers ----
        QT = const.tile([128, V * 4, MB], F32, name="QT")   # own rows
        YTg = [const.tile([128, V * 4, 512], F32, name=f"YT{g}") for g in range(NG)]
        mxs = const.tile([128, NG, V * 4], F32, name="mxs")

        pools = (xpool, ypool, sqpool, sspool, trp, identF, epsb)

        # ---- own rows -> QT ----
        for qc in range(NQ):
            _process_chunk(nc, pools, xq_d[128 * qc : 128 * (qc + 1)], QT,
                           128 * qc, nc.vector)
        QTr = QT.bitcast(BF16).rearrange("p vd (j t) -> p vd t j", t=2)

        # ---- stream groups ----
        for g in range(NG):
            for c4 in range(4):
                gc = 4 * g + c4
                _process_chunk(
                    nc, pools, x_d[128 * gc : 128 * (gc + 1)], YTg[g],
                    128 * c4, nc.vector)
            YTr = YTg[g].bitcast(BF16).rearrange("p vd (j t) -> p vd t j", t=2)
            for v in range(V):
                for mc in range(4):
                    acc = accp.tile([128, 512], F32, tag="acc", name="acc")
                    for ph in range(8):
                        d2, t = ph // 2, ph % 2
                        nc.tensor.matmul(
                            acc[:],
                            QTr[:, v * 4 + d2, t, 128 * mc : 128 * (mc + 1)],
                            YTr[:, v * 4 + d2, t, :],
                            start=(ph == 0), stop=False)
                    nc.tensor.matmul(
                        acc[:], gselI[:, g, :],
                        zpat[:, 384 - 128 * mc : 896 - 128 * mc],
                        start=False, stop=True)
                    nc.vector.reduce_max(
                        mxs[:, g, v * 4 + mc : v * 4 + mc + 1], acc[:],
                        axis=mybir.AxisListType.X)

        # ---- finale ----
        fm = const.tile([128, V * 4], F32, name="fm")
        nc.vector.reduce_max(
            fm[:], mxs.rearrange("p g c -> p c g"), axis=mybir.AxisListType.X)
        tt = const.tile([128, V * 4], F32, name="tt")
        nc.vector.tensor_scalar(
            tt[:], fm[:], -2.0, 2.0, mybir.AluOpType.mult, mybir.AluOpType.add)
        lg = const.tile([128, V * 4], F32, name="lg")
        nc.scalar.activation(lg[:], tt[:], AF.Ln, bias=epsb[:])
        ps2 = smallp.tile([1, V * 4], F32, tag="sps", name="ps2")
        nc.tensor.matmul(ps2[:], ones[:], lg[:], start=True, stop=True)
        tot = const.tile([1, 1], F32, name="tot")
        nc.vector.reduce_sum(tot[:], ps2[:], axis=mybir.AxisListType.X)
        tots = const.tile([1, 1], F32, name="tots")
        nc.vector.tensor_scalar_mul(tots[:], tot[:], -0.5 / B)
        nc.sync.dma_start(out_d, tots[:])

    nc.compile()  # bacc legalization: wait-splitting (<=1/instr), regs, fusion
    return nc


_CACHED = {}


def _run(x, trace=False):
    x = np.ascontiguousarray(np.asarray(x, dtype=np.float32))
    assert x.shape == (B, V, D), x.shape
    if "nc" not in _CACHED:
        _CACHED["nc"] = build()
    nc = _CACHED["nc"]
    in_maps = []
    for r in range(NCORES):
        gsel = np.zeros((1, NG), np.float32)
        gsel[0, r] = 1.0
        in_maps.append({
            "x": x,
            "xq": np.ascontiguousarray(x[MB * r : MB * (r + 1)]),
            "gsel": np.broadcast_to(gsel, (128, NG)).copy(),
        })
    res = bass_utils.run_bass_kernel_spmd(
        nc, in_maps, core_ids=list(range(NCORES)), trace=trace)
    partials = [np.float32(res.results[r]["out"][0, 0]) for r in range(NCORES)]
    total = np.float32(np.sum(np.array(partials, dtype=np.float32)))
    return total, res


def kernel(student_global_cls_tokens):
    total, _ = _run(student_global_cls_tokens, trace=False)
    return np.asarray(total, dtype=np.float32)



# revision 2
# speedup vs baseline: 1.8962x; 1.8962x over previous
"""KoLeo-loss kernel for Trainium2, 8 NeuronCores — symmetric-Gram version.

Math: rows are L2-normalized; for unit vectors dist(a,b) = sqrt(2-2*a.b), so
the per-row NN distance needs only the row-max of the diagonal-masked cosine
Gram matrix.  G = Y Y^T is symmetric: each off-diagonal 512x512 group-block
computed ONCE yields row-max partials for its row-group (free-dim reduce) and
col-max partials for its col-group (partition-dim reduce via GPSIMD
partition_all_reduce).  This halves both matmul FLOPs and HBM traffic vs
computing full Gram rows per core.

Work split (groups = 8 x 512 rows): 28 off-diagonal group pairs + 8 diagonal
triangles.  Each core gets 4 off-units {r0,r1}x{c0,c1} (32 slots for 28 pairs:
4 pairs duplicated, harmless for max) + the masked diagonal triangle of r0
(chunk pairs a<=b, 10 of 16 chunk-blocks).  Every core runs the IDENTICAL
program; the host packs that core's 4 groups into its `xg` input in processing
order (r0, c0, c1, r1) and maps the partial-max outputs back.

Per-core pipeline:
  - stream 16 chunks [128, 2, 1024] fp32; ACT square+accum -> ss;
    ACT sqrt(ss+eps); DVE reciprocal -> rinv; DVE scale -> y bf16
  - PE transpose (fp32 pair-packed bf16) -> YT[slot] d-major in SBUF
    (8 transposes/chunk into one 2-bank PSUM tile, ONE ACT copyback)
  - bf16 matmuls K=1024 in 8 phases into PSUM [128, 2, 512] tiles;
    diag chunk-blocks get a -4*I mask matmul (N=128) as the stop op
  - DVE reduce_max rows; DVE scalar_tensor_tensor max-merge cols into cmx;
    GPSIMD partition_all_reduce(max) -> col partials
  - outputs: rm [128, V,2,4] row partials, cm [1, V,3,512] col partials;
    host combines maxes across cores and takes -log(sqrt(2-2m)+eps) mean.
"""

import os
import sys
from contextlib import ExitStack

import numpy as np

sys.path.insert(0, "/opt/trn_rl_repo")

import concourse.bass as bass
import concourse.mybir as mybir
import concourse.tile as tile
from concourse import bacc, bass_isa, bass_utils

F32 = mybir.dt.float32
BF16 = mybir.dt.bfloat16
AF = mybir.ActivationFunctionType
ALU = mybir.AluOpType

B, V, D = 4096, 2, 1024
NCORES = 8
G, GS, NCH = 8, 512, 4   # groups, group size, 128-chunks per group
EPS = 1e-8
MASKV = -4.0

# per-core groups (r0, r1, c0, c1); r0 also carries the masked diagonal
# triangle.  Off-units {r0,r1}x{c0,c1} of all cores cover all 28 group
# pairs; the r0s cover all 8 diagonals.  (Verified in session notes.)
CORES = [
    (0, 1, 2, 3),
    (1, 0, 4, 5),
    (6, 7, 0, 1),
    (2, 3, 4, 5),
    (3, 2, 6, 7),
    (4, 5, 6, 7),
    (5, 0, 1, 4),
    (7, 2, 3, 6),
]
# xg slot packing order (processing order): r0, c0, c1, r1
SLOT_OF = (0, 3, 1, 2)  # (r0, r1, c0, c1) -> slot index in xg


def _front_chunk(nc, pools, xg_d, s, c, YT, identF, epsb):
    """Load chunk c of slot s, normalize, transpose into YT[s][:, :, c*128:]."""
    xpool, ypool, sqpool, sspool, trp = pools
    xt = xpool.tile([128, V, D], F32, tag="xraw", name="xraw")
    nc.sync.dma_start(xt[:], xg_d[s, 128 * c : 128 * (c + 1)])

    ss = sspool.tile([128, V], F32, tag="ss", name="ss")
    sq = sqpool.tile([128, D], BF16, tag="sq", name="sq")
    for v in range(V):
        nc.scalar.activation(sq[:], xt[:, v, :], AF.Square, accum_out=ss[:, v : v + 1])
    nrm = sspool.tile([128, V], F32, tag="nrm", name="nrm")
    nc.scalar.activation(nrm[:], ss[:], AF.Sqrt, bias=epsb[:])
    rinv = sspool.tile([128, V], F32, tag="rinv", name="rinv")
    nc.vector.reciprocal(rinv[:], nrm[:])

    yb = ypool.tile([128, V, D], BF16, tag="ybf", name="ybf")
    for v in range(V):
        nc.vector.tensor_scalar_mul(yb[:, v, :], xt[:, v, :], rinv[:, v : v + 1])

    ybF = yb.bitcast(F32)  # [128, V, 512]: word j packs y[.., 2j], y[.., 2j+1]
    tp = trp.tile([128, V * 4, 128], F32, tag="tp", name="tp")
    for v in range(V):
        for d2 in range(4):
            nc.tensor.transpose(
                tp[:, v * 4 + d2, :], ybF[:, v, 128 * d2 : 128 * (d2 + 1)], identF[:])
    nc.scalar.copy(YT[s][:, :, 128 * c : 128 * (c + 1)], tp[:])


def build():
    nc = bacc.Bacc("TRN2", debug=False)
    xg_d = nc.dram_tensor("xg", [4, GS, V, D], F32, kind="ExternalInput").ap()
    rm_d = nc.dram_tensor("rm", [128, V * 2 * NCH], F32, kind="ExternalOutput").ap()
    cm_d = nc.dram_tensor("cm", [1, V * 3 * GS], F32, kind="ExternalOutput").ap()

    with ExitStack() as ctx:
        tc = ctx.enter_context(tile.TileContext(nc))
        const = ctx.enter_context(tc.tile_pool(name="const", bufs=1))
        xpool = ctx.enter_context(tc.tile_pool(name="xpool", bufs=3))
        ypool = ctx.enter_context(tc.tile_pool(name="ypool", bufs=2))
        sqpool = ctx.enter_context(tc.tile_pool(name="sqpool", bufs=2))
        sspool = ctx.enter_context(tc.tile_pool(name="sspool", bufs=2))
        accp = ctx.enter_context(tc.tile_pool(name="accp", bufs=2, space="PSUM"))
        trp = ctx.enter_context(tc.tile_pool(name="trp", bufs=2, space="PSUM"))

        # ---- constants ----
        identF = const.tile([128, 128], F32, name="identF")
        nc.gpsimd.memset(identF[:], 0.0)
        nc.gpsimd.affine_select(
            out=identF[:], in_=identF[:], compare_op=ALU.not_equal,
            fill=1.0, base=0, pattern=[[-1, 128]], channel_multiplier=1)

        identB = const.tile([128, 128], BF16, name="identB")
        nc.gpsimd.memset(identB[:], 0.0)
        nc.gpsimd.affine_select(
            out=identB[:], in_=identB[:], compare_op=ALU.not_equal,
            fill=1.0, base=0, pattern=[[-1, 128]], channel_multiplier=1)

        negI = const.tile([128, 128], BF16, name="negI")
        nc.gpsimd.memset(negI[:], 0.0)
        nc.gpsimd.affine_select(
            out=negI[:], in_=negI[:], compare_op=ALU.not_equal,
            fill=MASKV, base=0, pattern=[[-1, 128]], channel_multiplier=1)

        epsb = const.tile([128, 1], F32, name="epsb")
        nc.gpsimd.memset(epsb[:], EPS)

        # Dummy transpose: advances PE's observed gpsimd clock past the const
        # writes so real transposes need only their one data wait.
        tpd = trp.tile([128, V * 4, 128], F32, tag="tp", name="tpd")
        nc.tensor.transpose(tpd[:, 0, :], identF[:], identF[:])

        # ---- persistent buffers ----
        YT = [const.tile([128, V * 4, GS], F32, name=f"YT{s}") for s in range(4)]
        # rmcoll[p, v, ri, chunk, contrib]: contribs 0/1 = off-units c0/c1,
        # 2 = diag triangle (ri==0 only)
        rmcoll = const.tile([128, V, 2, NCH, 3], F32, name="rmcoll")
        nc.vector.memset(rmcoll[:], MASKV)
        # cmx[p, v, ci, :]: running col-max; ci 0/1 = c-slots, 2 = diag (r0)
        cmx = const.tile([128, V, 3, GS], F32, name="cmx")
        nc.vector.memset(cmx[:], MASKV)
        cmr = const.tile([128, V, 3, GS], F32, name="cmr")

        pools = (xpool, ypool, sqpool, sspool, trp)
        YTr = [None] * 4

        def front_slot(s):
            for c in range(NCH):
                _front_chunk(nc, pools, xg_d, s, c, YT, identF, epsb)
            YTr[s] = YT[s].bitcast(BF16).rearrange("p vd (j t) -> p vd t j", t=2)

        def off_unit(sa, sb, ri, ci):
            """Gram block rows=slot sa (r-index ri), cols=slot sb (c-index ci)."""
            for v in range(V):
                for pair in range(2):
                    acc = accp.tile([128, 2, GS], F32, tag="acc", name="acc")
                    for sub in range(2):
                        mc = 2 * pair + sub
                        for ph in range(8):
                            d2, t = ph // 2, ph % 2
                            nc.tensor.matmul(
                                acc[:, sub, :],
                                YTr[sa][:, v * 4 + d2, t, 128 * mc : 128 * (mc + 1)],
                                YTr[sb][:, v * 4 + d2, t, :],
                                start=(ph == 0), stop=(ph == 7))
                    nc.vector.reduce_max(
                        rmcoll[:, v, ri, 2 * pair : 2 * pair + 2, ci],
                        acc[:], axis=mybir.AxisListType.X)
                    for sub in range(2):
                        nc.vector.scalar_tensor_tensor(
                            cmx[:, v, ci, :], acc[:, sub, :], 1.0,
                            cmx[:, v, ci, :], ALU.mult, ALU.max)

        def diag_unit(s0):
            """Masked diagonal triangle of slot s0: chunk pairs a<=b."""
            for v in range(V):
                for half in range(2):
                    acc = accp.tile([128, 2, GS], F32, tag="acc", name="acc")
                    for sub in range(2):
                        a = 2 * half + sub
                        N = (NCH - a) * 128
                        for ph in range(8):
                            d2, t = ph // 2, ph % 2
                            nc.tensor.matmul(
                                acc[:, sub, :N],
                                YTr[s0][:, v * 4 + d2, t, 128 * a : 128 * (a + 1)],
                                YTr[s0][:, v * 4 + d2, t, 128 * a :],
                                start=(ph == 0), stop=False)
                        # -4*I on the self block (cols 0:128 of this slice)
                        nc.tensor.matmul(
                            acc[:, sub, 0:128], negI[:], identB[:],
                            start=False, stop=True)
                        nc.vector.reduce_max(
                            rmcoll[:, v, 0, a, 2:3], acc[:, sub, :N],
                            axis=mybir.AxisListType.X)
                    for sub in range(2):
                        a = 2 * half + sub
                        for b in range(a + 1, NCH):
                            nc.vector.scalar_tensor_tensor(
                                cmx[:, v, 2, 128 * b : 128 * (b + 1)],
                                acc[:, sub, 128 * (b - a) : 128 * (b - a + 1)],
                                1.0,
                                cmx[:, v, 2, 128 * b : 128 * (b + 1)],
                                ALU.mult, ALU.max)

        def col_allreduce(ci):
            for v in range(V):
                nc.gpsimd.partition_all_reduce(
                    cmr[:, v, ci, :], cmx[:, v, ci, :],
                    channels=128, reduce_op=bass_isa.ReduceOp.max)

        # ---- schedule ----
        # slots: 0=r0, 1=c0, 2=c1, 3=r1; load order r0, c0, r1, c1 so that
        # c0's col partials finish before the last unit (allreduce overlaps).
        front_slot(0)
        diag_unit(0)
        col_allreduce(2)
        front_slot(1)
        off_unit(0, 1, 0, 0)
        front_slot(3)
        off_unit(3, 1, 1, 0)
        col_allreduce(0)
        front_slot(2)
        off_unit(0, 2, 0, 1)
        off_unit(3, 2, 1, 1)
        col_allreduce(1)

        # ---- finale: fold contribs, write outputs ----
        rmfin = const.tile([128, V, 2, NCH], F32, name="rmfin")
        nc.vector.reduce_max(rmfin[:], rmcoll[:], axis=mybir.AxisListType.X)
        nc.sync.dma_start(rm_d, rmfin.rearrange("p v r c -> p (v r c)"))
        nc.sync.dma_start(cm_d, cmr[0:1].rearrange("p v i j -> p (v i j)"))

    nc.compile()
    return nc


_CACHED = {}


def _run(x, trace=False):
    x = np.ascontiguousarray(np.asarray(x, dtype=np.float32))
    assert x.shape == (B, V, D), x.shape
    if "nc" not in _CACHED:
        _CACHED["nc"] = build()
    nc = _CACHED["nc"]
    xr = x.reshape(G, GS, V, D)
    in_maps = []
    for r0, r1, c0, c1 in CORES:
        in_maps.append({"xg": np.ascontiguousarray(xr[[r0, c0, c1, r1]])})
    res = bass_utils.run_bass_kernel_spmd(
        nc, in_maps, core_ids=list(range(NCORES)), trace=trace)

    M = np.full((V, B), MASKV, np.float32)
    for k, (r0, r1, c0, c1) in enumerate(CORES):
        rm = np.asarray(res.results[k]["rm"]).reshape(128, V, 2, NCH)
        cm = np.asarray(res.results[k]["cm"]).reshape(V, 3, GS)
        for v in range(V):
            for ri, g in ((0, r0), (1, r1)):
                seg = M[v, g * GS : (g + 1) * GS]
                np.maximum(seg, rm[:, v, ri, :].T.reshape(GS), out=seg)
            for ci, g in ((0, c0), (1, c1), (2, r0)):
                seg = M[v, g * GS : (g + 1) * GS]
                np.maximum(seg, cm[v, ci], out=seg)

    m = M.astype(np.float64)
    dist = np.sqrt(np.maximum(2.0 - 2.0 * m, 0.0))
    total = np.float32(-np.sum(np.mean(np.log(dist + EPS), axis=1)))
    return total, res


def kernel(student_global_cls_tokens):
    total, _ = _run(student_global_cls_tokens, trace=False)
    return np.asarray(total, dtype=np.float32)
